# revision 51
# baseline (speedup 1.0000x reference)
"""Trainium2 Bass kernel for nn_AffinityDiffProposedModel (v2).

Reference model (B=4, L=256, D=512, H=8, DH=64):
  Q/K/V = relu(x @ W + b); euclidean diff-attention per head
  (logits = -||q-k||/sqrt(DH)), softmax over keys, query-mask,
  ctx @ W_bil @ keys^T + b_bil -> (B, L, L).

Sharding: 8 cores = 4 batches x 2 query-halves (128 query rows each).
Each core computes its (128, 256) slice of the output.

v2 design notes:
  * All matmul operands are bf16 (1 row/cycle at any free size, half the
    HBM traffic); PSUM accumulates in f32. rel-err gate is 2e-2; this
    lands ~4e-3.
  * Host packs transposed inputs (qsT/kbT/vbT) and chunk-major weights,
    so no PE transposes are needed on the way in. The query mask is a
    host-computed column.
  * The distance matrix is built TRANSPOSED, d2T[k, q], so exp() writes
    attn^T directly and the per-head attention needs no transposes:
      d2T = -2*(qk^T - k2[k]/2 - q2[q]/2)
    with the k2/q2 terms folded into the same PSUM accumulation group as
    K=64 matmuls against a constant -0.5 operand (the all-constant side
    broadcasts the contraction of the other side). Sqrt then needs no
    bias and runs per-head; exactly 2 ACT table loads (Sqrt, Exp) total.
  * HW constraint (found by probing): matmuls whose stationary operands
    sit at different partition bases (head-even at 0, head-odd at 64)
    fault the NEFF if they target the same PSUM bank -> each head's d2T
    gets its own PSUM tile.
  * Softmax normalization is deferred: ctx_un = pT^T @ [V | 1] puts the
    softmax row-sum s in PSUM column 64; ctx = ctx_un * (1/s * qm) fuses
    into the per-head PSUM->SBUF copy (query-mask included).
  * Bilinear tail: scores = ctxT^T @ M with M = Wb @ keys^T computed
    early (overlaps the softmax phase); each 128-wide ctx chunk is
    PE-transposed and folded into the open scores accumulation as soon
    as its two heads finish, so the tail after the last exp is short.
    b_bil rides into the scores PSUM as a K=1 matmul.
  * 5 input DMAs total (packed blobs, both HWDGE queues), ordered so
    first-needed bytes stream first.
"""

import os
import sys

import numpy as np

B, L, D, H = 4, 256, 512, 8
DH = 64
LQ = 128  # query rows per core
NC = 4  # D // 128 feature chunks
NR = 2  # L // 128 key-row chunks
N_CORES = 8

_REPO = "/opt/trn_rl_repo"


def _ensure_path():
    if _REPO not in sys.path:
        sys.path.insert(0, _REPO)


def build_nc():
    _ensure_path()
    import concourse.bacc as bacc
    import concourse.mybir as mybir
    import concourse.tile as tile

    nc = bacc.Bacc("TRN2", target_bir_lowering=False, debug=False, num_devices=N_CORES)

    f32 = mybir.dt.float32
    bf16 = mybir.dt.bfloat16

    # ---- DRAM I/O ----
    # early inputs [128, 1674] bf16: kbT | qsT | ident | bqT|bkT|qm
    blobA = nc.dram_tensor("blobA", [128, 1682], bf16, kind="ExternalInput").ap()
    # later inputs [128, 1920] bf16: vbT | (p0 rows: ones | bv | bbil)
    blobB = nc.dram_tensor("blobB", [128, 1920], bf16, kind="ExternalInput").ap()
    Wk = nc.dram_tensor("Wk", [128, NC, D], bf16, kind="ExternalInput").ap()
    Wq = nc.dram_tensor("Wq", [128, NC, D], bf16, kind="ExternalInput").ap()
    # Wv | WbT packed
    Wvb = nc.dram_tensor("Wvb", [128, 2, NC, D], bf16, kind="ExternalInput").ap()
    out = nc.dram_tensor("out", [LQ, L], f32, kind="ExternalOutput").ap()

    with tile.TileContext(nc) as tc:
        _body(nc, tc, mybir, blobA, blobB, Wk, Wq, Wvb, out)
    nc.compile()
    return nc


def _body(nc, tc, mybir, blobA, blobB, Wk, Wq, Wvb, out):
    from contextlib import ExitStack

    f32 = mybir.dt.float32
    bf16 = mybir.dt.bfloat16
    Alu = mybir.AluOpType
    Act = mybir.ActivationFunctionType

    ctx = ExitStack()
    with ctx:
        const = ctx.enter_context(tc.tile_pool(name="const", bufs=1))
        persist = ctx.enter_context(tc.tile_pool(name="persist", bufs=1))
        dists = ctx.enter_context(tc.tile_pool(name="dists", bufs=2))
        ps_proj = ctx.enter_context(tc.tile_pool(name="ps_proj", bufs=2, space="PSUM"))
        ps_pair = ctx.enter_context(tc.tile_pool(name="ps_pair", bufs=3, space="PSUM"))
        ps_ctx = ctx.enter_context(tc.tile_pool(name="ps_ctx", bufs=2, space="PSUM"))
        ps_tp = ctx.enter_context(tc.tile_pool(name="ps_tp", bufs=1, space="PSUM"))

        # ---- input loads: split by need-time, issue on both queues ----
        blobA_t = persist.tile([128, 1682], bf16, tag="blobA")
        nc.sync.dma_start(out=blobA_t, in_=blobA)
        Wk_t = persist.tile([128, NC, D], bf16, tag="wk")
        nc.scalar.dma_start(out=Wk_t, in_=Wk)
        Wq_t = persist.tile([128, NC, D], bf16, tag="wq")
        nc.scalar.dma_start(out=Wq_t, in_=Wq)
        Wvb_t = persist.tile([128, 2, NC, D], bf16, tag="wvb")
        nc.scalar.dma_start(out=Wvb_t, in_=Wvb)
        blobB_t = persist.tile([128, 1920], bf16, tag="blobB")
        nc.sync.dma_start(out=blobB_t, in_=blobB)

        kbT_sb = blobA_t[:, 0:1024].rearrange("p (c l) -> p c l", c=NC)
        qsT_sb = blobA_t[:, 1024:1536].rearrange("p (c l) -> p c l", c=NC)
        ident_sb = blobA_t[:, 1536:1664]
        vbT_sb = blobB_t[:, 0:1024].rearrange("p (c l) -> p c l", c=NC)
        ones_sb = blobB_t[0:1, 1024:1152]
        bv_sb = blobB_t[0:1, 1152:1664]
        bbil_row = blobB_t[0:1, 1664:1920]
        Wk_sb = [Wk_t[:, kc, :] for kc in range(NC)]
        Wq_sb = [Wq_t[:, kc, :] for kc in range(NC)]
        Wv_sb = [Wvb_t[:, 0, kc, :] for kc in range(NC)]
        Wb_sb = [Wvb_t[:, 1, kc, :] for kc in range(NC)]
        cstf = blobA_t[:, 1664:1682].bitcast(f32)
        bqT_sb = cstf[:, 0:NC]
        bkT_sb = cstf[:, NC:2 * NC]
        qm_sb = cstf[:, 2 * NC:2 * NC + 1]

        # ---- persistent compute tiles ----
        KT = persist.tile([128, NC, L], bf16, tag="KT")
        QT = persist.tile([128, NC, LQ], bf16, tag="QT")
        sqk = persist.tile([128, NC, L], bf16, tag="sqk")
        sq2 = persist.tile([128, NC, 2, LQ], bf16, tag="sq2")
        nhalf = persist.tile([128, L], bf16, tag="nhalf")
        pT_all = persist.tile([128, H, NR, LQ], bf16, tag="pT_all")
        Vaug = persist.tile([128, NR, H, DH + 1], bf16, tag="vaug")
        ctxN = persist.tile([128, D], bf16, tag="ctxN")
        ctxT = persist.tile([128, NC, LQ], bf16, tag="ctxT")
        M_sb = persist.tile([128, NC, L], bf16, tag="M_sb")
        rs = persist.tile([128, H], f32, tag="rs")
        out_sb = persist.tile([128, L], f32, tag="out_sb")

        nc.gpsimd.memset(Vaug[:, :, :, DH:DH + 1], 1.0)
        nc.gpsimd.memset(nhalf, -0.5)


        # ---- projection phase: all K chunks first (Wk arrives before
        # Wq, and the PE issues in order — a Qproj stall would block the
        # remaining Kproj chunks), then all Q chunks ----
        for c in range(NC):
            cs = slice(c * 128, (c + 1) * 128)
            pk_t = ps_proj.tile([128, D], f32, tag="proj")
            pk = pk_t[:, 0:L]
            for kc in range(NC):
                nc.tensor.matmul(pk, Wk_sb[kc][:, cs], kbT_sb[:, kc, :],
                                 start=(kc == 0), stop=(kc == NC - 1))
            nc.vector.tensor_scalar(out=KT[:, c, :], in0=pk,
                                    scalar1=bkT_sb[:, c:c + 1], scalar2=0.0,
                                    op0=Alu.add, op1=Alu.max)
            nc.gpsimd.tensor_mul(sqk[:, c, :], KT[:, c, :], KT[:, c, :])
        for c in range(NC):
            cs = slice(c * 128, (c + 1) * 128)
            pq_t = ps_proj.tile([128, D], f32, tag="proj")
            pq = pq_t[:, 0:LQ]
            for kc in range(NC):
                nc.tensor.matmul(pq, Wq_sb[kc][:, cs], qsT_sb[:, kc, :],
                                 start=(kc == 0), stop=(kc == NC - 1))
            nc.vector.tensor_scalar(out=QT[:, c, :], in0=pq,
                                    scalar1=bqT_sb[:, c:c + 1], scalar2=0.0,
                                    op0=Alu.add, op1=Alu.max)
            for i in range(2):
                nc.vector.tensor_mul(sq2[:, c, i, :], QT[:, c, :], QT[:, c, :])

        # ---- dist phase: transposed dist^2 per head + sqrt ----
        dist_tiles = []
        for c in range(NC):
            if c % 2 == 0:
                dquad = dists.tile([128, 2, 2, NR * LQ], bf16, tag="dist")
                dist_tiles.append(dquad)
            dpair = dist_tiles[c // 2][:, c % 2, :, :]
            for j in range(2):
                hs = slice(64 * j, 64 * j + 64)
                d2 = ps_pair.tile([128, NR, LQ], f32, tag="d2")
                # -q2/2 for both key chunks in one matmul (sq replicated),
                # opening the accumulation group over the whole region
                nc.tensor.matmul(d2.rearrange("p a b -> p (a b)"),
                                 nhalf[hs, 0:128],
                                 sq2[hs, c, :, :].rearrange("p a b -> p (a b)"),
                                 start=True, stop=False)
                for rc in range(NR):
                    rcs = slice(rc * 128, (rc + 1) * 128)
                    nc.tensor.matmul(d2[:, rc, :], KT[hs, c, rcs],
                                     QT[hs, c, :], start=False, stop=False)
                    nc.tensor.matmul(d2[:, rc, :], sqk[hs, c, rcs],
                                     nhalf[hs, 0:LQ], start=False,
                                     stop=(rc == NR - 1))
                nc.scalar.activation(out=dpair[:, j, :],
                                     in_=d2.rearrange("p a b -> p (a b)"),
                                     func=Act.Sqrt, scale=-2.0)

        # ---- V projection (natural layout, bias rides as K=1 matmul) ----
        for rc in range(NR):
            rcs = slice(rc * 128, (rc + 1) * 128)
            pv = ps_proj.tile([128, D], f32, tag="proj")
            for kc in range(NC):
                nc.tensor.matmul(pv, vbT_sb[:, kc, rcs], Wv_sb[kc],
                                 start=(kc == 0), stop=False)
            nc.tensor.matmul(pv, ones_sb, bv_sb, start=False, stop=True)
            nc.vector.tensor_scalar(out=Vaug[:, rc, :, 0:DH],
                                    in0=pv.rearrange("p (h e) -> p h e", h=H),
                                    scalar1=0.0, scalar2=None, op0=Alu.max)

        # ---- M = Wb @ keys^T (bilinear right factor; independent of
        # attention, computed here to overlap with the softmax phase) ----
        for dc in range(NC):
            dcs = slice(dc * 128, (dc + 1) * 128)
            pm_t = ps_proj.tile([128, D], f32, tag="proj")
            pm = pm_t[:, 0:L]
            for ec in range(NC):
                nc.tensor.matmul(pm, Wb_sb[ec][:, dcs], kbT_sb[:, ec, :],
                                 start=(ec == 0), stop=(ec == NC - 1))
            nc.vector.tensor_copy(out=M_sb[:, dc, :], in_=pm)

        # ---- exp (four heads per activation; free-running — the extra
        # mid-phase table reload hides in ACT idle gaps, and early heads'
        # ctx work overlaps the remaining distance phase) ----
        for cp in range(NC // 2):
            pslice = pT_all[:, 4 * cp:4 * cp + 4, :, :]
            nc.scalar.activation(
                out=pslice.rearrange("p a b c -> p (a b c)"),
                in_=dist_tiles[cp].rearrange("p a b c -> p (a b c)"),
                func=Act.Exp, scale=-0.125)

        # ---- per-head ctx (+ row-sum via augmented ones column),
        # with the scores accumulation folded in per 128-chunk ----
        ops_t = ps_proj.tile([128, D], f32, tag="proj")
        ops = ops_t[:, 0:L]
        for hp in range(H // 2):
            cps2 = ps_ctx.tile([128, 2, DH + 1], f32, tag="cps")
            for i in range(2):
                h = 2 * hp + i
                cps = cps2[:, i, :]
                for rc in range(NR):
                    nc.tensor.matmul(cps, pT_all[:, h, rc, :],
                                     Vaug[:, rc, h, :],
                                     start=(rc == 0), stop=(rc == NR - 1))
                nc.vector.reciprocal(out=rs[:, h:h + 1], in_=cps[:, DH:DH + 1])
                nc.vector.tensor_scalar(out=ctxN[:, h * DH:(h + 1) * DH],
                                        in0=cps[:, 0:DH],
                                        scalar1=rs[:, h:h + 1], op0=Alu.mult,
                                        scalar2=qm_sb, op1=Alu.mult)
            # this 128-wide ctx chunk is complete: transpose it and fold
            # it into the open scores accumulation right away
            dc = hp
            tp_t = ps_tp.tile([128, LQ], bf16, tag="tp")
            nc.tensor.transpose(tp_t, ctxN[:, dc * 128:(dc + 1) * 128],
                                ident_sb)
            nc.vector.tensor_copy(out=ctxT[:, dc, :], in_=tp_t)
            nc.tensor.matmul(ops, ctxT[:, dc, :], M_sb[:, dc, :],
                             start=(dc == 0), stop=False)
            if dc == NC - 1:
                # + b_bil, riding as a K=1 matmul closing the group
                nc.tensor.matmul(ops, ones_sb, bbil_row,
                                 start=False, stop=True)

        nc.scalar.copy(out=out_sb[0:64, :], in_=ops[0:64, :])
        nc.vector.tensor_copy(out=out_sb[64:128, :], in_=ops[64:128, :])
        nc.scalar.dma_start(out=out[0:64, :], in_=out_sb[0:64, :])
        nc.sync.dma_start(out=out[64:128, :], in_=out_sb[64:128, :])


_CONSTS = None


def _consts():
    global _CONSTS
    if _CONSTS is None:
        import ml_dtypes
        _CONSTS = {
            "ident": np.eye(128, dtype=np.float32).astype(ml_dtypes.bfloat16),
        }
    return _CONSTS


_NC_CACHE = None


def _get_nc():
    global _NC_CACHE
    if _NC_CACHE is None:
        _NC_CACHE = build_nc()
    return _NC_CACHE


def _bf(x):
    import ml_dtypes
    return np.ascontiguousarray(x).astype(ml_dtypes.bfloat16)


def _pack_T(x, free):
    """[rows, D] -> transposed chunk-major [128, NC, rows] (bf16)."""
    import ml_dtypes
    xT = np.ascontiguousarray(x.T)  # [D, rows]
    return np.ascontiguousarray(
        xT.reshape(NC, 128, free).transpose(1, 0, 2)).astype(ml_dtypes.bfloat16)


def _pack_W(w):
    """[D, D] -> chunk-major [128, NC, D] (bf16)."""
    import ml_dtypes
    return np.ascontiguousarray(
        w.reshape(NC, 128, D).transpose(1, 0, 2)).astype(ml_dtypes.bfloat16)


def make_in_maps(queries, keys, values, Wq, bq, Wk, bk, Wv, bv, W_bil, b_bil):
    c = _consts()
    f = lambda x: np.asarray(x, dtype=np.float32)
    queries, keys, values = f(queries), f(keys), f(values)
    wvb = np.stack([_pack_W(f(Wv)),
                    _pack_W(np.ascontiguousarray(f(W_bil).T))], axis=1)
    shared = {
        "Wk": _pack_W(f(Wk)), "Wq": _pack_W(f(Wq)),
        "Wvb": np.ascontiguousarray(wvb),
    }
    qmask = (np.abs(queries.sum(-1)) != 0.0).astype(np.float32)  # (B, L)
    in_maps = []
    for core in range(N_CORES):
        b, qh = divmod(core, 2)
        rows = slice(qh * LQ, (qh + 1) * LQ)
        m = dict(shared)
        import ml_dtypes
        blobA_m = np.zeros((128, 1682), ml_dtypes.bfloat16)
        blobA_m[:, 0:1024] = _pack_T(keys[b], L).reshape(128, 1024)
        blobA_m[:, 1024:1536] = _pack_T(queries[b, rows, :], LQ).reshape(128, 512)
        blobA_m[:, 1536:1664] = c["ident"]
        cstf = np.zeros((128, 9), np.float32)
        cstf[:, 0:NC] = f(bq).reshape(NC, 128).T
        cstf[:, NC:2 * NC] = f(bk).reshape(NC, 128).T
        cstf[:, 2 * NC] = qmask[b, rows]
        blobA_m[:, 1664:1682] = cstf.view(ml_dtypes.bfloat16)
        blobB_m = np.zeros((128, 1920), ml_dtypes.bfloat16)
        blobB_m[:, 0:1024] = _pack_T(values[b], L).reshape(128, 1024)
        blobB_m[0, 1024:1152] = 1.0
        blobB_m[0, 1152:1664] = f(bv).astype(ml_dtypes.bfloat16)
        blobB_m[0, 1664:1920] = f(b_bil)[0]
        m["blobA"] = blobA_m
        m["blobB"] = blobB_m
        in_maps.append(m)
    return in_maps


def kernel(**inputs):
    _ensure_path()
    from concourse.bass_utils import run_bass_kernel_spmd

    nc = _get_nc()
    in_maps = make_in_maps(**inputs)
    trace = os.environ.get("KERNEL_TRACE", "0") == "1"
    res = run_bass_kernel_spmd(nc, in_maps, core_ids=list(range(N_CORES)),
                               trace=trace)
    if trace:
        kernel.last_result = res
    out = np.zeros((B, L, L), np.float32)
    for core in range(N_CORES):
        b, qh = divmod(core, 2)
        out[b, qh * LQ:(qh + 1) * LQ, :] = res.results[core]["out"]
    return out


# revision 52
# speedup vs baseline: 1.0075x; 1.0075x over previous
"""Trainium2 Bass kernel for nn_AffinityDiffProposedModel (v2).

Reference model (B=4, L=256, D=512, H=8, DH=64):
  Q/K/V = relu(x @ W + b); euclidean diff-attention per head
  (logits = -||q-k||/sqrt(DH)), softmax over keys, query-mask,
  ctx @ W_bil @ keys^T + b_bil -> (B, L, L).

Sharding: 8 cores = 4 batches x 2 query-halves (128 query rows each).
Each core computes its (128, 256) slice of the output.

v2 design notes:
  * All matmul operands are bf16 (1 row/cycle at any free size, half the
    HBM traffic); PSUM accumulates in f32. rel-err gate is 2e-2; this
    lands ~4e-3.
  * Host packs transposed inputs (qsT/kbT/vbT) and chunk-major weights,
    so no PE transposes are needed on the way in. The query mask is a
    host-computed column.
  * The distance matrix is built TRANSPOSED, d2T[k, q], so exp() writes
    attn^T directly and the per-head attention needs no transposes:
      d2T = -2*(qk^T - k2[k]/2 - q2[q]/2)
    with the k2/q2 terms folded into the same PSUM accumulation group as
    K=64 matmuls against a constant -0.5 operand (the all-constant side
    broadcasts the contraction of the other side). Sqrt then needs no
    bias and runs per-head; exactly 2 ACT table loads (Sqrt, Exp) total.
  * HW constraint (found by probing): matmuls whose stationary operands
    sit at different partition bases (head-even at 0, head-odd at 64)
    fault the NEFF if they target the same PSUM bank -> each head's d2T
    gets its own PSUM tile.
  * Softmax normalization is deferred: ctx_un = pT^T @ [V | 1] puts the
    softmax row-sum s in PSUM column 64; ctx = ctx_un * (1/s * qm) fuses
    into the per-head PSUM->SBUF copy (query-mask included).
  * Bilinear tail: scores = ctxT^T @ M with M = Wb @ keys^T computed
    early (overlaps the softmax phase); each 128-wide ctx chunk is
    PE-transposed and folded into the open scores accumulation as soon
    as its two heads finish, so the tail after the last exp is short.
    b_bil rides into the scores PSUM as a K=1 matmul.
  * 5 input DMAs total (packed blobs, both HWDGE queues), ordered so
    first-needed bytes stream first.
"""

import os
import sys

import numpy as np

B, L, D, H = 4, 256, 512, 8
DH = 64
LQ = 128  # query rows per core
NC = 4  # D // 128 feature chunks
NR = 2  # L // 128 key-row chunks
N_CORES = 8

_REPO = "/opt/trn_rl_repo"


def _ensure_path():
    if _REPO not in sys.path:
        sys.path.insert(0, _REPO)


def build_nc():
    _ensure_path()
    import concourse.bacc as bacc
    import concourse.mybir as mybir
    import concourse.tile as tile

    nc = bacc.Bacc("TRN2", target_bir_lowering=False, debug=False, num_devices=N_CORES)

    f32 = mybir.dt.float32
    bf16 = mybir.dt.bfloat16

    # ---- DRAM I/O ----
    # early inputs [128, 1674] bf16: kbT | qsT | ident | bqT|bkT|qm
    blobA = nc.dram_tensor("blobA", [128, 1682], bf16, kind="ExternalInput").ap()
    # later inputs [128, 1920] bf16: vbT | (p0 rows: ones | bv | bbil)
    blobB = nc.dram_tensor("blobB", [128, 1920], bf16, kind="ExternalInput").ap()
    Wk = nc.dram_tensor("Wk", [128, NC, D], bf16, kind="ExternalInput").ap()
    Wq = nc.dram_tensor("Wq", [128, NC, D], bf16, kind="ExternalInput").ap()
    # Wv | WbT packed
    Wvb = nc.dram_tensor("Wvb", [128, 2, NC, D], bf16, kind="ExternalInput").ap()
    out = nc.dram_tensor("out", [LQ, L], f32, kind="ExternalOutput").ap()

    with tile.TileContext(nc) as tc:
        _body(nc, tc, mybir, blobA, blobB, Wk, Wq, Wvb, out)
    nc.compile()
    return nc


def _body(nc, tc, mybir, blobA, blobB, Wk, Wq, Wvb, out):
    from contextlib import ExitStack

    f32 = mybir.dt.float32
    bf16 = mybir.dt.bfloat16
    Alu = mybir.AluOpType
    Act = mybir.ActivationFunctionType

    ctx = ExitStack()
    with ctx:
        const = ctx.enter_context(tc.tile_pool(name="const", bufs=1))
        persist = ctx.enter_context(tc.tile_pool(name="persist", bufs=1))
        dists = ctx.enter_context(tc.tile_pool(name="dists", bufs=2))
        ps_proj = ctx.enter_context(tc.tile_pool(name="ps_proj", bufs=2, space="PSUM"))
        ps_pair = ctx.enter_context(tc.tile_pool(name="ps_pair", bufs=3, space="PSUM"))
        ps_ctx = ctx.enter_context(tc.tile_pool(name="ps_ctx", bufs=2, space="PSUM"))
        ps_tp = ctx.enter_context(tc.tile_pool(name="ps_tp", bufs=1, space="PSUM"))

        # ---- input loads: split by need-time, issue on both queues ----
        blobA_t = persist.tile([128, 1682], bf16, tag="blobA")
        nc.sync.dma_start(out=blobA_t, in_=blobA)
        Wk_t = persist.tile([128, NC, D], bf16, tag="wk")
        nc.scalar.dma_start(out=Wk_t, in_=Wk)
        Wq_t = persist.tile([128, NC, D], bf16, tag="wq")
        nc.scalar.dma_start(out=Wq_t, in_=Wq)
        Wvb_t = persist.tile([128, 2, NC, D], bf16, tag="wvb")
        nc.scalar.dma_start(out=Wvb_t, in_=Wvb)
        blobB_t = persist.tile([128, 1920], bf16, tag="blobB")
        nc.sync.dma_start(out=blobB_t, in_=blobB)

        kbT_sb = blobA_t[:, 0:1024].rearrange("p (c l) -> p c l", c=NC)
        qsT_sb = blobA_t[:, 1024:1536].rearrange("p (c l) -> p c l", c=NC)
        ident_sb = blobA_t[:, 1536:1664]
        vbT_sb = blobB_t[:, 0:1024].rearrange("p (c l) -> p c l", c=NC)
        ones_sb = blobB_t[0:1, 1024:1152]
        bv_sb = blobB_t[0:1, 1152:1664]
        bbil_row = blobB_t[0:1, 1664:1920]
        Wk_sb = [Wk_t[:, kc, :] for kc in range(NC)]
        Wq_sb = [Wq_t[:, kc, :] for kc in range(NC)]
        Wv_sb = [Wvb_t[:, 0, kc, :] for kc in range(NC)]
        Wb_sb = [Wvb_t[:, 1, kc, :] for kc in range(NC)]
        cstf = blobA_t[:, 1664:1682].bitcast(f32)
        bqT_sb = cstf[:, 0:NC]
        bkT_sb = cstf[:, NC:2 * NC]
        qm_sb = cstf[:, 2 * NC:2 * NC + 1]

        # ---- persistent compute tiles ----
        KT = persist.tile([128, NC, L], bf16, tag="KT")
        QT = persist.tile([128, NC, LQ], bf16, tag="QT")
        sqk = persist.tile([128, NC, L], bf16, tag="sqk")
        sq2 = persist.tile([128, NC, 2, LQ], bf16, tag="sq2")
        nhalf = persist.tile([128, L], bf16, tag="nhalf")
        pT_all = persist.tile([128, H, NR, LQ], bf16, tag="pT_all")
        Vaug = persist.tile([128, NR, H, DH + 1], bf16, tag="vaug")
        ctxN = persist.tile([128, D], bf16, tag="ctxN")
        ctxT = persist.tile([128, NC, LQ], bf16, tag="ctxT")
        M_sb = persist.tile([128, NC, L], bf16, tag="M_sb")
        rs = persist.tile([128, H], f32, tag="rs")
        out_sb = persist.tile([128, L], f32, tag="out_sb")

        nc.gpsimd.memset(Vaug[:, :, :, DH:DH + 1], 1.0)
        nc.gpsimd.memset(nhalf, -0.5)


        # ---- projection phase: all K chunks first (Wk arrives before
        # Wq, and the PE issues in order — a Qproj stall would block the
        # remaining Kproj chunks), then all Q chunks ----
        for c in range(NC):
            cs = slice(c * 128, (c + 1) * 128)
            pk_t = ps_proj.tile([128, D], f32, tag="proj")
            pk = pk_t[:, 0:L]
            for kc in range(NC):
                nc.tensor.matmul(pk, Wk_sb[kc][:, cs], kbT_sb[:, kc, :],
                                 start=(kc == 0), stop=(kc == NC - 1))
            nc.vector.tensor_scalar(out=KT[:, c, :], in0=pk,
                                    scalar1=bkT_sb[:, c:c + 1], scalar2=0.0,
                                    op0=Alu.add, op1=Alu.max)
            nc.gpsimd.tensor_mul(sqk[:, c, :], KT[:, c, :], KT[:, c, :])
        for c in range(NC):
            cs = slice(c * 128, (c + 1) * 128)
            pq_t = ps_proj.tile([128, D], f32, tag="proj")
            pq = pq_t[:, 0:LQ]
            for kc in range(NC):
                nc.tensor.matmul(pq, Wq_sb[kc][:, cs], qsT_sb[:, kc, :],
                                 start=(kc == 0), stop=(kc == NC - 1))
            nc.vector.tensor_scalar(out=QT[:, c, :], in0=pq,
                                    scalar1=bqT_sb[:, c:c + 1], scalar2=0.0,
                                    op0=Alu.add, op1=Alu.max)
            for i in range(2):
                nc.vector.tensor_mul(sq2[:, c, i, :], QT[:, c, :], QT[:, c, :])

        # ---- dist phase: transposed dist^2 per head + sqrt ----
        dist_tiles = []
        for c in range(NC):
            if c % 2 == 0:
                dquad = dists.tile([128, 2, 2, NR * LQ], bf16, tag="dist")
                dist_tiles.append(dquad)
            dpair = dist_tiles[c // 2][:, c % 2, :, :]
            for j in range(2):
                hs = slice(64 * j, 64 * j + 64)
                d2 = ps_pair.tile([128, NR, LQ], f32, tag="d2")
                # -q2/2 for both key chunks in one matmul (sq replicated),
                # opening the accumulation group over the whole region
                nc.tensor.matmul(d2.rearrange("p a b -> p (a b)"),
                                 nhalf[hs, 0:128],
                                 sq2[hs, c, :, :].rearrange("p a b -> p (a b)"),
                                 start=True, stop=False)
                for rc in range(NR):
                    rcs = slice(rc * 128, (rc + 1) * 128)
                    nc.tensor.matmul(d2[:, rc, :], KT[hs, c, rcs],
                                     QT[hs, c, :], start=False, stop=False)
                    nc.tensor.matmul(d2[:, rc, :], sqk[hs, c, rcs],
                                     nhalf[hs, 0:LQ], start=False,
                                     stop=(rc == NR - 1))
                nc.scalar.activation(out=dpair[:, j, :],
                                     in_=d2.rearrange("p a b -> p (a b)"),
                                     func=Act.Sqrt, scale=-2.0)

        # ---- V projection (natural layout, bias rides as K=1 matmul) ----
        for rc in range(NR):
            rcs = slice(rc * 128, (rc + 1) * 128)
            pv = ps_proj.tile([128, D], f32, tag="proj")
            for kc in range(NC):
                nc.tensor.matmul(pv, vbT_sb[:, kc, rcs], Wv_sb[kc],
                                 start=(kc == 0), stop=False)
            nc.tensor.matmul(pv, ones_sb, bv_sb, start=False, stop=True)
            nc.vector.tensor_scalar(out=Vaug[:, rc, :, 0:DH],
                                    in0=pv.rearrange("p (h e) -> p h e", h=H),
                                    scalar1=0.0, scalar2=None, op0=Alu.max)

        # ---- M = Wb @ keys^T (bilinear right factor; independent of
        # attention, computed here to overlap with the softmax phase) ----
        for dc in range(NC):
            dcs = slice(dc * 128, (dc + 1) * 128)
            pm_t = ps_proj.tile([128, D], f32, tag="proj")
            pm = pm_t[:, 0:L]
            for ec in range(NC):
                nc.tensor.matmul(pm, Wb_sb[ec][:, dcs], kbT_sb[:, ec, :],
                                 start=(ec == 0), stop=(ec == NC - 1))
            nc.vector.tensor_copy(out=M_sb[:, dc, :], in_=pm)

        # ---- exp (four heads per activation; scheduled after every sqrt
        # so the ACT engine loads each table exactly once) ----
        with tc.tile_wait_until(0.03):
            for cp in range(NC // 2):
                pslice = pT_all[:, 4 * cp:4 * cp + 4, :, :]
                nc.scalar.activation(
                    out=pslice.rearrange("p a b c -> p (a b c)"),
                    in_=dist_tiles[cp].rearrange("p a b c -> p (a b c)"),
                    func=Act.Exp, scale=-0.125)

        # ---- per-head ctx (+ row-sum via augmented ones column),
        # with the scores accumulation folded in per 128-chunk ----
        ops_t = ps_proj.tile([128, D], f32, tag="proj")
        ops = ops_t[:, 0:L]
        for hp in range(H // 2):
            cps2 = ps_ctx.tile([128, 2, DH + 1], f32, tag="cps")
            for i in range(2):
                h = 2 * hp + i
                cps = cps2[:, i, :]
                for rc in range(NR):
                    nc.tensor.matmul(cps, pT_all[:, h, rc, :],
                                     Vaug[:, rc, h, :],
                                     start=(rc == 0), stop=(rc == NR - 1))
                nc.vector.reciprocal(out=rs[:, h:h + 1], in_=cps[:, DH:DH + 1])
                nc.vector.tensor_scalar(out=ctxN[:, h * DH:(h + 1) * DH],
                                        in0=cps[:, 0:DH],
                                        scalar1=rs[:, h:h + 1], op0=Alu.mult,
                                        scalar2=qm_sb, op1=Alu.mult)
            # this 128-wide ctx chunk is complete: transpose it and fold
            # it into the open scores accumulation right away
            dc = hp
            tp_t = ps_tp.tile([128, LQ], bf16, tag="tp")
            nc.tensor.transpose(tp_t, ctxN[:, dc * 128:(dc + 1) * 128],
                                ident_sb)
            nc.vector.tensor_copy(out=ctxT[:, dc, :], in_=tp_t)
            nc.tensor.matmul(ops, ctxT[:, dc, :], M_sb[:, dc, :],
                             start=(dc == 0), stop=False)
            if dc == NC - 1:
                # + b_bil, riding as a K=1 matmul closing the group
                nc.tensor.matmul(ops, ones_sb, bbil_row,
                                 start=False, stop=True)

        nc.scalar.copy(out=out_sb[0:64, :], in_=ops[0:64, :])
        nc.vector.tensor_copy(out=out_sb[64:128, :], in_=ops[64:128, :])
        nc.scalar.dma_start(out=out[0:64, :], in_=out_sb[0:64, :])
        nc.sync.dma_start(out=out[64:128, :], in_=out_sb[64:128, :])


_CONSTS = None


def _consts():
    global _CONSTS
    if _CONSTS is None:
        import ml_dtypes
        _CONSTS = {
            "ident": np.eye(128, dtype=np.float32).astype(ml_dtypes.bfloat16),
        }
    return _CONSTS


_NC_CACHE = None


def _get_nc():
    global _NC_CACHE
    if _NC_CACHE is None:
        _NC_CACHE = build_nc()
    return _NC_CACHE


def _bf(x):
    import ml_dtypes
    return np.ascontiguousarray(x).astype(ml_dtypes.bfloat16)


def _pack_T(x, free):
    """[rows, D] -> transposed chunk-major [128, NC, rows] (bf16)."""
    import ml_dtypes
    xT = np.ascontiguousarray(x.T)  # [D, rows]
    return np.ascontiguousarray(
        xT.reshape(NC, 128, free).transpose(1, 0, 2)).astype(ml_dtypes.bfloat16)


def _pack_W(w):
    """[D, D] -> chunk-major [128, NC, D] (bf16)."""
    import ml_dtypes
    return np.ascontiguousarray(
        w.reshape(NC, 128, D).transpose(1, 0, 2)).astype(ml_dtypes.bfloat16)


def make_in_maps(queries, keys, values, Wq, bq, Wk, bk, Wv, bv, W_bil, b_bil):
    c = _consts()
    f = lambda x: np.asarray(x, dtype=np.float32)
    queries, keys, values = f(queries), f(keys), f(values)
    wvb = np.stack([_pack_W(f(Wv)),
                    _pack_W(np.ascontiguousarray(f(W_bil).T))], axis=1)
    shared = {
        "Wk": _pack_W(f(Wk)), "Wq": _pack_W(f(Wq)),
        "Wvb": np.ascontiguousarray(wvb),
    }
    qmask = (np.abs(queries.sum(-1)) != 0.0).astype(np.float32)  # (B, L)
    in_maps = []
    for core in range(N_CORES):
        b, qh = divmod(core, 2)
        rows = slice(qh * LQ, (qh + 1) * LQ)
        m = dict(shared)
        import ml_dtypes
        blobA_m = np.zeros((128, 1682), ml_dtypes.bfloat16)
        blobA_m[:, 0:1024] = _pack_T(keys[b], L).reshape(128, 1024)
        blobA_m[:, 1024:1536] = _pack_T(queries[b, rows, :], LQ).reshape(128, 512)
        blobA_m[:, 1536:1664] = c["ident"]
        cstf = np.zeros((128, 9), np.float32)
        cstf[:, 0:NC] = f(bq).reshape(NC, 128).T
        cstf[:, NC:2 * NC] = f(bk).reshape(NC, 128).T
        cstf[:, 2 * NC] = qmask[b, rows]
        blobA_m[:, 1664:1682] = cstf.view(ml_dtypes.bfloat16)
        blobB_m = np.zeros((128, 1920), ml_dtypes.bfloat16)
        blobB_m[:, 0:1024] = _pack_T(values[b], L).reshape(128, 1024)
        blobB_m[0, 1024:1152] = 1.0
        blobB_m[0, 1152:1664] = f(bv).astype(ml_dtypes.bfloat16)
        blobB_m[0, 1664:1920] = f(b_bil)[0]
        m["blobA"] = blobA_m
        m["blobB"] = blobB_m
        in_maps.append(m)
    return in_maps


def kernel(**inputs):
    _ensure_path()
    from concourse.bass_utils import run_bass_kernel_spmd

    nc = _get_nc()
    in_maps = make_in_maps(**inputs)
    trace = os.environ.get("KERNEL_TRACE", "0") == "1"
    res = run_bass_kernel_spmd(nc, in_maps, core_ids=list(range(N_CORES)),
                               trace=trace)
    if trace:
        kernel.last_result = res
    out = np.zeros((B, L, L), np.float32)
    for core in range(N_CORES):
        b, qh = divmod(core, 2)
        out[b, qh * LQ:(qh + 1) * LQ, :] = res.results[core]["out"]
    return out
